# revision 1
# baseline (speedup 1.0000x reference)
"""Contrastive-learning loss kernel for Trainium2 (8 NeuronCores, Bass/Tile).

Problem (hardcoded shapes): B=16, L=512, DIN1=256, DIN2=192, DH=256, DF=128.
  emb1 = MLP_a(feature1); emb2 = MLP_b(feature2)          # (B, L, DF)
  positive = rowdot(f1, f2) + band-mean terms              # (N,)  N = B*L = 8192
  negative = logsumexp(f1 @ f2.T, axis=-1) - log N         # (N,)
  loss = mean(-positive + negative)

Sharding: data-parallel over B for embeddings/positives (2 batches per core);
the N x N negatives matrix is sharded row-wise. Each core computes the full
emb2 from a column-ROTATED copy of feature2 (its own batches first), so the
device program is identical across cores (pure SPMD, no partition-id): the
local rows are always columns [0, 1024) and logsumexp is invariant to column
order.

v2 pipeline (vs the v1 ACT-only drain): exp+rowsum of the sim tiles is the
per-core steady-state wall, so it is split three ways per column group:
  - ACT: activation(Exp, accum_out) straight out of PSUM (5 of 8 row tiles)
  - DVE: Schraudolph fast-exp: y16 = int16(A*x + B) so that y16's bf16 bit
    pattern ~= exp(x) (max rel err ~4%, mean ~0), then a bf16 tensor_reduce
    row-sum (2 of the 3 offloaded tiles)
  - Pool: the third offloaded tile's row-sum via scalar_tensor_tensor's
    fused accum_out
MLP2's relu+bias and the PSUM->SBUF e2 copy+bias run on GPSIMD (Pool) to
keep DVE free for exp; positives' elementwise work is spread across groups.
Band masks arrive prescaled by 1/count so the banded-mean rescale is free:
  pos = e1.(e2 + W1' + W2t') + e2.W2s'
The PE order interleaves neg-tile fills with next-group MLP2 chunks so the
tensor engine never idles long enough for HAM to re-throttle the PE clock.
"""

import numpy as np

import concourse.bacc as bacc
import concourse.tile as tile
from concourse import mybir
from concourse.bass_utils import run_bass_kernel_spmd
from concourse.masks import make_identity

F32 = mybir.dt.float32
F32R = mybir.dt.float32r
BF16 = mybir.dt.bfloat16
I16 = mybir.dt.int16

B, L, DIN1, DIN2, DH, DF = 16, 512, 256, 192, 256, 128
N = B * L            # 8192 total rows
NCORES = 8
NB = B // NCORES     # 2 local batches per core
NLOC = NB * L        # 1024 local rows per core
NT = NLOC // 128     # 8 local row tiles
NEG_FD = 1024        # columns exp'd per drain instruction
NGRP = N // NEG_FD   # 8 column groups

# Schraudolph fast-exp in int16/bf16: bf16_bits(exp(x)) ~= int16(A2*x + B2)
# (output cast truncates; sigma tuned for ~zero mean rel err, max ~4%)
EXP_A2 = float(2.0**7 / np.log(2.0))
EXP_B2 = float(127.0 * 2.0**7 - 6.833)

# Per-group drain of the 8 row tiles: 'a' = ACT exp+accum, 'd' = DVE
# convert + DVE bf16 reduce. (GPSIMD can neither read PSUM nor reduce, so
# it only gets the positives' SBUF-only elementwise work; MLP2's relu runs
# fused on ACT and the e2 copy+bias on DVE.)
# Measured: ACT tile 1.27us, DVE tile 2.30us, ACT relu 0.63, DVE copy 0.69.
# Min-makespan: 5 ACT tiles + 3 DVE tiles, relus on ACT, one copy on each.
DRAIN = ['a', 'a', 'd', 'a', 'd', 'a', 'd', 'a']


def _build(share_tgt: bool):
    nc = bacc.Bacc("TRN2", target_bir_lowering=False, debug=False)

    x1t_d = nc.dram_tensor("x1t", [DIN1, NLOC], BF16, kind="ExternalInput")
    x2t_d = nc.dram_tensor("x2t", [DIN2, N], BF16, kind="ExternalInput")
    w1a_d = nc.dram_tensor("w1a", [DIN1, DH], BF16, kind="ExternalInput")
    w2a_d = nc.dram_tensor("w2a", [DH, DF], F32R, kind="ExternalInput")
    w1b_d = nc.dram_tensor("w1b", [DIN2, DH], BF16, kind="ExternalInput")
    w2b_d = nc.dram_tensor("w2b", [DH, DF], F32R, kind="ExternalInput")
    b1a_d = nc.dram_tensor("b1a", [128, 2], F32, kind="ExternalInput")
    b2a_d = nc.dram_tensor("b2a", [128, 1], F32, kind="ExternalInput")
    b1b_d = nc.dram_tensor("b1b", [128, 2], F32, kind="ExternalInput")
    b2b_d = nc.dram_tensor("b2b", [128, 1], F32, kind="ExternalInput")
    # band masks prescaled by 1/count(j) along j
    bms_d = nc.dram_tensor("bms", [L, L], BF16, kind="ExternalInput")
    if not share_tgt:
        bmt_d = nc.dram_tensor("bmt", [L, L], BF16, kind="ExternalInput")
    pos_d = nc.dram_tensor("pos_out", [128, NT], F32, kind="ExternalOutput")
    se_d = nc.dram_tensor("se_out", [128, NT], F32, kind="ExternalOutput")

    with tile.TileContext(nc) as tc:
        import contextlib

        with contextlib.ExitStack() as stack:
            const = stack.enter_context(tc.tile_pool(name="const", bufs=1))
            big = stack.enter_context(tc.tile_pool(name="big", bufs=1))
            h2pool = stack.enter_context(tc.tile_pool(name="h2pool", bufs=3))
            posp = stack.enter_context(tc.tile_pool(name="posp", bufs=1))
            convp = stack.enter_context(tc.tile_pool(name="convp", bufs=3))

            # ---- constants / inputs (ordered so MLP1 can start ASAP) ----
            ident = const.tile([128, 128], F32)
            make_identity(nc, ident[:])

            w1a = const.tile([128, 2, DH], BF16)
            nc.sync.dma_start(
                out=w1a[:], in_=w1a_d.ap().rearrange("(t p) m -> p t m", p=128)
            )
            x1t = big.tile([128, 2, NLOC], BF16)
            for cc in range(2):
                nc.sync.dma_start(
                    out=x1t[:, :, cc * 512 : (cc + 1) * 512],
                    in_=x1t_d.ap().rearrange("(t p) c -> p t c", p=128)[
                        :, :, cc * 512 : (cc + 1) * 512
                    ],
                )
            w2a = const.tile([128, 2, DF], F32R)
            nc.sync.dma_start(
                out=w2a[:], in_=w2a_d.ap().rearrange("(t p) m -> p t m", p=128)
            )
            b1a = const.tile([128, 2], F32)
            nc.sync.dma_start(out=b1a[:], in_=b1a_d.ap())
            b2a = const.tile([128, 1], F32)
            nc.sync.dma_start(out=b2a[:], in_=b2a_d.ap())

            w1b_a = const.tile([128, DH], BF16)
            nc.sync.dma_start(out=w1b_a[:], in_=w1b_d.ap()[0:128, :])
            w1b_b = const.tile([64, DH], BF16)
            nc.sync.dma_start(out=w1b_b[:], in_=w1b_d.ap()[128:192, :])
            w2b = const.tile([128, 2, DF], F32R)
            nc.sync.dma_start(
                out=w2b[:], in_=w2b_d.ap().rearrange("(t p) m -> p t m", p=128)
            )
            b1b = const.tile([128, 2], F32)
            nc.sync.dma_start(out=b1b[:], in_=b1b_d.ap())
            b2b = const.tile([128, 1], F32)
            nc.sync.dma_start(out=b2b[:], in_=b2b_d.ap())

            # stream feature2^T (bf16), local cols first; the first chunk
            # feeds the pre-loop MLP2 chunks, then the band masks, then rest
            x2a = big.tile([128, N], BF16)
            x2b = big.tile([64, N], BF16)
            nc.sync.dma_start(out=x2a[:, 0:2048], in_=x2t_d.ap()[0:128, 0:2048])
            nc.sync.dma_start(out=x2b[:, 0:2048], in_=x2t_d.ap()[128:DIN2, 0:2048])

            bms = const.tile([128, 4, L], BF16)
            nc.sync.dma_start(
                out=bms[:], in_=bms_d.ap().rearrange("(t p) j -> p t j", p=128)
            )
            if share_tgt:
                bmt = bms
            else:
                bmt = const.tile([128, 4, L], BF16)
                nc.sync.dma_start(
                    out=bmt[:], in_=bmt_d.ap().rearrange("(t p) j -> p t j", p=128)
                )

            for g in range(1, 4):
                cs = slice(g * 2048, (g + 1) * 2048)
                nc.sync.dma_start(out=x2a[:, cs], in_=x2t_d.ap()[0:128, cs])
                nc.sync.dma_start(out=x2b[:, cs], in_=x2t_d.ap()[128:DIN2, cs])

            e1t = big.tile([128, NLOC], F32R)
            e2t = big.tile([128, N], F32R)
            h1t = big.tile([128, 2, NLOC], F32R)
            e1nat = big.tile([128, NT, DF], BF16)
            e2nat = big.tile([128, NT, DF], BF16)
            w1nat = big.tile([128, NT, DF], F32)
            w2snat = big.tile([128, NT, DF], F32)
            w2tnat = w2snat if share_tgt else big.tile([128, NT, DF], F32)
            pos_all = big.tile([128, NT], F32)
            acc_all = big.tile([128, NT * NGRP], F32)
            se_all = big.tile([128, NT], F32)
            # Pool STT reduce needs an in1 of zeros and a junk out
            zero_bf = const.tile([128, NEG_FD], BF16)
            nc.gpsimd.memset(zero_bf[:], 0.0)
            junk_bf = big.tile([128, NEG_FD], BF16)

            psA = stack.enter_context(tc.tile_pool(name="psumA", bufs=1, space="PSUM"))

            # ---- PE warm-up: get HAM to 8/8 while input DMAs stream ----
            zr_l = const.tile([128, 128], BF16)
            nc.gpsimd.memset(zr_l[:], 0.0)
            zr_r = const.tile([128, 512], BF16)
            nc.gpsimd.memset(zr_r[:], 0.0)
            warm_ps = psA.tile([128, 512], F32, tag="sps", bufs=2)
            for _ in range(8):
                nc.tensor.matmul(warm_ps[:], zr_l[:], zr_r[:], start=True, stop=True)

            # ---- MLP1: h1T = relu(W1a^T @ x1T + b1a); e1T = W2a^T @ h1T + b2a
            for cc in range(2):
                cols = slice(cc * 512, (cc + 1) * 512)
                h1ps = psA.tile([128, 2, 512], F32, tag="hps", bufs=1)
                for mt in range(2):
                    for kt in range(2):
                        nc.tensor.matmul(
                            h1ps[:, mt, :],
                            w1a[:, kt, mt * 128 : (mt + 1) * 128],
                            x1t[:, kt, cols],
                            start=(kt == 0),
                            stop=(kt == 1),
                        )
                for mt in range(2):
                    nc.vector.tensor_scalar(
                        out=h1t[:, mt, cols],
                        in0=h1ps[:, mt, :],
                        scalar1=b1a[:, mt : mt + 1],
                        scalar2=0.0,
                        op0=mybir.AluOpType.add,
                        op1=mybir.AluOpType.max,
                    )
                e1ps = psA.tile([128, 512], F32, tag="sps", bufs=2)
                for kt in range(2):
                    nc.tensor.matmul(
                        e1ps[:],
                        w2a[:, kt, :],
                        h1t[:, kt, cols],
                        start=(kt == 0),
                        stop=(kt == 1),
                    )
                nc.vector.tensor_scalar_add(out=e1t[:, cols], in0=e1ps[:], scalar1=b2a[:])

            def transpose4(dst, srcT, tt, eng):
                # transpose 4 adjacent 128-col blocks into one PSUM bank,
                # then one 512-col copy out
                tp = psA.tile([128, 4, 128], F32, tag="sps", bufs=2, name=f"tp{tt}")
                for i in range(4):
                    t = 4 * tt + i
                    nc.tensor.transpose(
                        tp[:, i, :], srcT[:, t * 128 : (t + 1) * 128].bitcast(F32),
                        ident[:],
                    )
                eng.tensor_copy(dst[:, 4 * tt : 4 * tt + 4, :], tp[:])

            # banded averages (masks prescaled by 1/count):
            # W'[j,:] = sum_{|m-j|<=r} e[m,:] / cnt(j); 16 matmuls into one
            # PSUM bank per batch, one 512-col copy out
            def band_b(dst, bm, src, b, eng):
                wps = psA.tile([128, 4, 128], F32, tag="sps", bufs=2, name=f"wp{b}")
                for jt in range(4):
                    for mt in range(4):
                        nc.tensor.matmul(
                            wps[:, jt, :],
                            bm[:, mt, jt * 128 : (jt + 1) * 128],
                            src[:, 4 * b + mt, :],
                            start=(mt == 0),
                            stop=(mt == 3),
                        )
                eng.tensor_copy(dst[:, 4 * b : 4 * b + 4, :], wps[:])

            # e1-side work only needs MLP1 -> runs while x2 still streams in
            for tt in range(2):
                transpose4(e1nat, e1t, tt, nc.vector)

            # ---- MLP2 over all N tokens (relu + copy/bias on GPSIMD) ----
            def mlp2_chunk(ct):
                cols = slice(ct * 512, (ct + 1) * 512)
                h2ps = psA.tile([128, 2, 512], F32, tag="hps", bufs=1, name=f"h2ps{ct}")
                for mt in range(2):
                    msl = slice(mt * 128, (mt + 1) * 128)
                    nc.tensor.matmul(
                        h2ps[:, mt, :], w1b_a[:, msl], x2a[:, cols], start=True, stop=False
                    )
                    nc.tensor.matmul(
                        h2ps[:, mt, :], w1b_b[:, msl], x2b[:, cols], start=False, stop=True
                    )
                h2t = h2pool.tile([128, 2, 512], F32R, tag="h2t", name=f"h2t{ct}")
                for mt in range(2):
                    nc.vector.tensor_scalar(
                        out=h2t[:, mt, :],
                        in0=h2ps[:, mt, :],
                        scalar1=b1b[:, mt : mt + 1],
                        scalar2=0.0,
                        op0=mybir.AluOpType.add,
                        op1=mybir.AluOpType.max,
                    )
                e2ps = psA.tile([128, 512], F32, tag="sps", bufs=2, name=f"e2ps{ct}")
                for kt in range(2):
                    nc.tensor.matmul(
                        e2ps[:], w2b[:, kt, :], h2t[:, kt, :], start=(kt == 0), stop=(kt == 1)
                    )
                nc.vector.tensor_scalar_add(out=e2t[:, cols], in0=e2ps[:], scalar1=b2b[:])

            mlp2_chunk(0)
            mlp2_chunk(1)
            band_b(w1nat, bms, e1nat, 0, nc.vector)
            band_b(w1nat, bms, e1nat, 1, nc.vector)

            # positives pieces, spread across the group loop (Pool):
            # share: pos = e1.(e2 + W1' + W2s') + e2.W2s'
            # non-share: pos = e1.(e2 + W1' + W2t') + e2.W2s'
            pos_state = {}
            pos_jobs = []

            def make_pos_jobs():
                for b in range(NB):
                    bsl = slice(4 * b, 4 * b + 4)
                    ga = posp.tile([128, 4, DF], F32, tag=f"posg{b}")
                    gb = posp.tile([128, 4, DF], F32, tag=f"posh{b}")
                    r1 = posp.tile([128, 4], F32, tag=f"post{b}")
                    r2 = posp.tile([128, 4], F32, tag=f"posu{b}")
                    pos_state[b] = (ga, gb, r1, r2)

                    def j1(bsl=bsl, ga=ga):
                        nc.gpsimd.tensor_tensor(
                            out=ga[:], in0=w1nat[:, bsl, :], in1=w2tnat[:, bsl, :],
                            op=mybir.AluOpType.add,
                        )

                    def j2(bsl=bsl, ga=ga):
                        nc.gpsimd.tensor_tensor(
                            out=ga[:], in0=ga[:], in1=e2nat[:, bsl, :],
                            op=mybir.AluOpType.add,
                        )

                    def j3(bsl=bsl, ga=ga):
                        nc.gpsimd.tensor_tensor(
                            out=ga[:], in0=ga[:], in1=e1nat[:, bsl, :],
                            op=mybir.AluOpType.mult,
                        )

                    def j4(bsl=bsl, gb=gb):
                        nc.gpsimd.tensor_tensor(
                            out=gb[:], in0=w2snat[:, bsl, :], in1=e2nat[:, bsl, :],
                            op=mybir.AluOpType.mult,
                        )

                    pos_jobs.extend([j1, j2, j3, j4])

            make_pos_jobs()

            def emit_pos_piece():
                if pos_jobs:
                    pos_jobs.pop(0)()

            # ---- interleaved MLP2 + negatives, one 1024-col group at a time
            def neg_fill(t, g):
                lhs = e1t[:, t * 128 : (t + 1) * 128]
                np_ps = psA.tile([128, NEG_FD], F32, tag="neg", bufs=2, name=f"np{t}_{g}")
                for i in range(NEG_FD // 512):
                    c0 = g * NEG_FD + i * 512
                    nc.tensor.matmul(
                        np_ps[:, i * 512 : (i + 1) * 512],
                        lhs,
                        e2t[:, c0 : c0 + 512],
                        start=True,
                        stop=True,
                    )
                return np_ps

            def drain(np_ps, t, g):
                idx = t * NGRP + g
                kind = DRAIN[t]
                if kind == 'a':
                    nc.scalar.activation(
                        out=np_ps[:],
                        in_=np_ps[:],
                        func=mybir.ActivationFunctionType.Exp,
                        accum_out=acc_all[:, idx : idx + 1],
                    )
                else:
                    y16 = convp.tile([128, NEG_FD], I16, tag="y16", name=f"y{t}_{g}")
                    nc.vector.tensor_scalar(
                        out=y16[:],
                        in0=np_ps[:],
                        scalar1=EXP_A2,
                        scalar2=EXP_B2,
                        op0=mybir.AluOpType.mult,
                        op1=mybir.AluOpType.add,
                    )
                    if kind == 'd':
                        nc.vector.tensor_reduce(
                            out=acc_all[:, idx : idx + 1],
                            in_=y16[:].bitcast(BF16),
                            axis=mybir.AxisListType.X,
                            op=mybir.AluOpType.add,
                        )
                    else:  # 'p': Pool row-sum via fused accum
                        nc.gpsimd.scalar_tensor_tensor(
                            out=junk_bf[:],
                            in0=y16[:].bitcast(BF16),
                            scalar=1.0,
                            in1=zero_bf[:],
                            op0=mybir.AluOpType.mult,
                            op1=mybir.AluOpType.add,
                            accum_out=acc_all[:, idx : idx + 1],
                        )

            for g in range(NGRP):
                for t in range(4):
                    ps = neg_fill(t, g)
                    drain(ps, t, g)
                if g < NGRP - 1:
                    mlp2_chunk(2 * g + 2)
                for t in range(4, NT):
                    ps = neg_fill(t, g)
                    drain(ps, t, g)
                if g < NGRP - 1:
                    mlp2_chunk(2 * g + 3)
                if g == 0:
                    # e2-side transposes + bands (local cols = group 0)
                    for tt in range(2):
                        transpose4(e2nat, e2t, tt, nc.vector)
                    band_b(w2snat, bms, e2nat, 0, nc.vector)
                    band_b(w2snat, bms, e2nat, 1, nc.vector)
                    if not share_tgt:
                        band_b(w2tnat, bmt, e2nat, 0, nc.vector)
                        band_b(w2tnat, bmt, e2nat, 1, nc.vector)
                else:
                    emit_pos_piece()
                    emit_pos_piece()

            while pos_jobs:
                emit_pos_piece()

            # pos tail: reduces + final combine on DVE
            for b in range(NB):
                bsl = slice(4 * b, 4 * b + 4)
                ga, gb, r1, r2 = pos_state[b]
                nc.vector.tensor_reduce(
                    out=r1[:], in_=ga[:], axis=mybir.AxisListType.X, op=mybir.AluOpType.add
                )
                nc.vector.tensor_reduce(
                    out=r2[:], in_=gb[:], axis=mybir.AxisListType.X, op=mybir.AluOpType.add
                )
                nc.vector.tensor_add(pos_all[:, bsl], r1[:], r2[:])
            nc.sync.dma_start(out=pos_d.ap(), in_=pos_all[:])

            nc.vector.tensor_reduce(
                out=se_all[:],
                in_=acc_all[:].rearrange("p (t g) -> p t g", t=NT),
                axis=mybir.AxisListType.X,
                op=mybir.AluOpType.add,
            )
            nc.sync.dma_start(out=se_d.ap(), in_=se_all[:])

    nc.compile()
    return nc


_BUILD_CACHE: dict = {}


def _get_nc(share_tgt: bool):
    if share_tgt not in _BUILD_CACHE:
        _BUILD_CACHE[share_tgt] = _build(share_tgt)
    return _BUILD_CACHE[share_tgt]


def _band_mask_scaled(r: int) -> np.ndarray:
    """mask[m, j] = 1/cnt(j) if |m-j| <= r (inside [0,L)) else 0."""
    bm = np.zeros((L, L), dtype=np.float32)
    if r > 0:
        j = np.arange(L)
        lo = np.maximum(j - r, 0)
        hi = np.minimum(j + r + 1, L)
        m = np.arange(L)[:, None]
        bm = ((m >= lo[None, :]) & (m < hi[None, :])).astype(np.float32)
        bm /= (hi - lo).astype(np.float32)[None, :]
    return bm


def kernel(**inputs):
    loss, _ = _run(inputs, trace=False)
    return loss


def _run(inputs, trace=False, trace_kwargs=None):
    import ml_dtypes

    bf16 = ml_dtypes.bfloat16
    feature1 = inputs["feature1"]
    feature2 = inputs["feature2"]
    W1a, b1a, W2a, b2a = inputs["W1a"], inputs["b1a"], inputs["W2a"], inputs["b2a"]
    W1b, b1b, W2b, b2b = inputs["W1b"], inputs["b1b"], inputs["W2b"], inputs["b2b"]
    f1 = np.ascontiguousarray(np.asarray(feature1, dtype=np.float32))
    f2 = np.ascontiguousarray(np.asarray(feature2, dtype=np.float32))
    r_self = int(np.asarray(inputs["positive_range_self"]))
    r_tgt = int(np.asarray(inputs["positive_range_tgt"]))
    share_tgt = r_tgt == r_self

    nc = _get_nc(share_tgt)

    x2t_full = np.ascontiguousarray(f2.reshape(N, DIN2).T.astype(bf16))  # (192, 8192)
    common = {
        "w1a": np.ascontiguousarray(np.asarray(W1a, np.float32).astype(bf16)),
        "w2a": np.ascontiguousarray(np.asarray(W2a, np.float32)),
        "w1b": np.ascontiguousarray(np.asarray(W1b, np.float32).astype(bf16)),
        "w2b": np.ascontiguousarray(np.asarray(W2b, np.float32)),
        "b1a": np.ascontiguousarray(np.asarray(b1a, np.float32).reshape(2, 128).T),
        "b2a": np.asarray(b2a, np.float32).reshape(128, 1),
        "b1b": np.ascontiguousarray(np.asarray(b1b, np.float32).reshape(2, 128).T),
        "b2b": np.asarray(b2b, np.float32).reshape(128, 1),
        "bms": _band_mask_scaled(r_self).astype(bf16),
    }
    if not share_tgt:
        common["bmt"] = _band_mask_scaled(r_tgt).astype(bf16)

    in_maps = []
    for c in range(NCORES):
        x1t = np.ascontiguousarray(
            f1[c * NB : (c + 1) * NB].reshape(NLOC, DIN1).T.astype(bf16)
        )  # (256, 1024)
        # rotate feature2^T columns so this core's rows come first
        x2t = np.ascontiguousarray(
            np.concatenate(
                [x2t_full[:, c * NLOC :], x2t_full[:, : c * NLOC]], axis=1
            )
        )
        in_maps.append({**common, "x1t": x1t, "x2t": x2t})

    res = run_bass_kernel_spmd(
        nc,
        in_maps,
        core_ids=list(range(NCORES)),
        trace=trace,
        **(trace_kwargs or {}),
    )

    pos = np.empty(N, dtype=np.float64)
    se = np.empty(N, dtype=np.float64)
    for c in range(NCORES):
        # column t holds local rows [t*128, (t+1)*128) in partitions
        p = res.results[c]["pos_out"]  # (128, NT)
        s = res.results[c]["se_out"]
        pos[c * NLOC : (c + 1) * NLOC] = p.T.reshape(NLOC)
        se[c * NLOC : (c + 1) * NLOC] = s.T.reshape(NLOC)

    neg = np.log(se) - np.log(float(N))
    loss = np.mean(-pos + neg)
    return np.array(loss, dtype=np.float32), res



# revision 10
# speedup vs baseline: 1.1927x; 1.1927x over previous
"""Contrastive-learning loss kernel for Trainium2 (8 NeuronCores, Bass/Tile).

Problem (hardcoded shapes): B=16, L=512, DIN1=256, DIN2=192, DH=256, DF=128.
  emb1 = MLP_a(feature1); emb2 = MLP_b(feature2)          # (B, L, DF)
  positive = rowdot(f1, f2) + band-mean terms              # (N,)  N = B*L = 8192
  negative = logsumexp(f1 @ f2.T, axis=-1) - log N         # (N,)
  loss = mean(-positive + negative)

Sharding: data-parallel over B for embeddings/positives (2 batches per core);
the N x N negatives matrix is sharded row-wise. Each core computes the full
emb2 from a column-ROTATED copy of feature2 (its own batches first), so the
device program is identical across cores (pure SPMD, no partition-id): the
local rows are always columns [0, 1024) and logsumexp is invariant to column
order.

v3: the per-core wall is the PSUM drain of the sim tiles (only ACT and DVE
can read PSUM, both at ~1 elem/cycle/lane). Changes vs v2:
  - 3-way drain: ACT does exp with fused accum (4 tiles/group); DVE does the
    Schraudolph int16 convert only (4 tiles/group); GPSIMD picks up the bf16
    row-sums of the DVE tiles via tensor_scalar(accum_out=...) from SBUF
    (~1 cyc/elem, it cannot read PSUM but the y16 tiles are in SBUF).
  - MLP2's b1b is folded into the h2 matmul (x2b carries a ones row, w1b_b a
    bias row), so the h2 drain is ONE 1024-wide relu per chunk instead of two
    biased 512-wide tensor_scalars.
  - everything bf16 (weights, h, e) halves SBUF and enables FWL.
  - transposes/bands/pos pieces are staged as jobs consumed inside the group
    loop so the first neg fill starts right after MLP1 + 2 MLP2 chunks.
Band masks arrive prescaled by 1/count so the banded-mean rescale is free:
  pos = e1.(e2 + W1' + W2t') + e2.W2s'
"""

import numpy as np

import concourse.bacc as bacc
import concourse.tile as tile
from concourse import mybir
from concourse.bass_utils import run_bass_kernel_spmd
from concourse.masks import make_identity

F32 = mybir.dt.float32
BF16 = mybir.dt.bfloat16
I16 = mybir.dt.int16

B, L, DIN1, DIN2, DH, DF = 16, 512, 256, 192, 256, 128
N = B * L            # 8192 total rows
NCORES = 8
NB = B // NCORES     # 2 local batches per core
NLOC = NB * L        # 1024 local rows per core
NT = NLOC // 128     # 8 local row tiles
NEG_FD = 1024        # columns drained per instruction
NGRP = N // NEG_FD   # 8 column groups

# Schraudolph fast-exp in int16/bf16: bf16_bits(exp(x)) ~= int16(A2*x + B2)
# (output cast truncates; sigma tuned for ~zero mean rel err, max ~4%)
EXP_A2 = float(2.0**7 / np.log(2.0))
EXP_B2 = float(127.0 * 2.0**7 - 6.833)

# Per-group drain of the 8 row tiles: 'a' = ACT exp + fused accum,
# 'd' = DVE Schraudolph convert; the two converts of a group PAIR are
# then combined with one bf16 2x-mode add + one reduce (per-elem reduce
# work halves vs a reduce per group).
DRAIN = ['a', 'd', 'a', 'a', 'd', 'a', 'd', 'a']


def _build(share_tgt: bool):
    nc = bacc.Bacc("TRN2", target_bir_lowering=False, debug=False)

    x1t_d = nc.dram_tensor("x1t", [DIN1, NLOC], BF16, kind="ExternalInput")
    x2t_d = nc.dram_tensor("x2t", [DIN2, N], BF16, kind="ExternalInput")
    w1a_d = nc.dram_tensor("w1a", [DIN1, DH], BF16, kind="ExternalInput")
    w2a_d = nc.dram_tensor("w2a", [DH, DF], BF16, kind="ExternalInput")
    w1b_d = nc.dram_tensor("w1b", [DIN2, DH], BF16, kind="ExternalInput")
    w2b_d = nc.dram_tensor("w2b", [DH, DF], BF16, kind="ExternalInput")
    b1a_d = nc.dram_tensor("b1a", [128, 2], F32, kind="ExternalInput")
    b2a_d = nc.dram_tensor("b2a", [128, 1], F32, kind="ExternalInput")
    b1b_d = nc.dram_tensor("b1b_row", [1, DH], BF16, kind="ExternalInput")
    b2b_d = nc.dram_tensor("b2b", [128, 1], F32, kind="ExternalInput")
    # band masks prescaled by 1/count(j)
    bms_d = nc.dram_tensor("bms", [L, L], BF16, kind="ExternalInput")
    if not share_tgt:
        bmt_d = nc.dram_tensor("bmt", [L, L], BF16, kind="ExternalInput")
    pos_d = nc.dram_tensor("pos_out", [128, NT], F32, kind="ExternalOutput")
    se_d = nc.dram_tensor("se_out", [128, NT], F32, kind="ExternalOutput")

    with tile.TileContext(nc) as tc:
        import contextlib

        with contextlib.ExitStack() as stack:
            const = stack.enter_context(tc.tile_pool(name="const", bufs=1))
            big = stack.enter_context(tc.tile_pool(name="big", bufs=1))
            h2pool = stack.enter_context(tc.tile_pool(name="h2pool", bufs=3))
            posp = stack.enter_context(tc.tile_pool(name="posp", bufs=1))
            convp = stack.enter_context(tc.tile_pool(name="convp", bufs=3))

            # ---- constants / inputs (ordered so MLP1 can start ASAP) ----
            ident = const.tile([128, 128], BF16)
            make_identity(nc, ident[:])

            # sync queue: MLP1's critical inputs first (x1t gates everything)
            x1t = big.tile([128, 2, NLOC], BF16)
            for cc in range(2):
                nc.sync.dma_start(
                    out=x1t[:, :, cc * 512 : (cc + 1) * 512],
                    in_=x1t_d.ap().rearrange("(t p) c -> p t c", p=128)[
                        :, :, cc * 512 : (cc + 1) * 512
                    ],
                )
            w1a = const.tile([128, 2, DH], BF16)
            nc.sync.dma_start(
                out=w1a[:], in_=w1a_d.ap().rearrange("(t p) m -> p t m", p=128)
            )
            b1a = const.tile([128, 2], F32)
            nc.sync.dma_start(out=b1a[:], in_=b1a_d.ap())
            w2a = const.tile([128, 2, DF], BF16)
            nc.sync.dma_start(
                out=w2a[:], in_=w2a_d.ap().rearrange("(t p) m -> p t m", p=128)
            )
            b2a = const.tile([128, 1], F32)
            nc.sync.dma_start(out=b2a[:], in_=b2a_d.ap())

            # scalar (ACT) HWDGE queue in parallel: MLP2 weights + x2 slab 0
            w1b_a = const.tile([128, DH], BF16)
            nc.scalar.dma_start(out=w1b_a[:], in_=w1b_d.ap()[0:128, :])
            # augmented: rows 0-63 = w1b[128:192], row 64 = b1b (ones-row trick)
            w1b_b = const.tile([65, DH], BF16)
            nc.scalar.dma_start(out=w1b_b[0:64, :], in_=w1b_d.ap()[128:192, :])
            nc.scalar.dma_start(out=w1b_b[64:65, :], in_=b1b_d.ap())
            w2b = const.tile([128, 2, DF], BF16)
            nc.scalar.dma_start(
                out=w2b[:], in_=w2b_d.ap().rearrange("(t p) m -> p t m", p=128)
            )
            b2b = const.tile([128, 1], F32)
            nc.scalar.dma_start(out=b2b[:], in_=b2b_d.ap())

            # stream feature2^T (bf16), local cols first; the first slab feeds
            # the pre-loop MLP2 chunks, then the band masks, then the rest
            x2a = big.tile([128, N], BF16)
            x2b = big.tile([65, N], BF16)
            nc.gpsimd.memset(x2b[64:65, :], 1.0)  # ones row: h2 += 1*b1b
            nc.scalar.dma_start(out=x2a[:, 0:2048], in_=x2t_d.ap()[0:128, 0:2048])
            nc.scalar.dma_start(out=x2b[0:64, 0:2048], in_=x2t_d.ap()[128:DIN2, 0:2048])

            bms = const.tile([128, 4, L], BF16)
            nc.scalar.dma_start(
                out=bms[:], in_=bms_d.ap().rearrange("(t p) j -> p t j", p=128)
            )
            if share_tgt:
                bmt = bms
            else:
                bmt = const.tile([128, 4, L], BF16)
                nc.scalar.dma_start(
                    out=bmt[:], in_=bmt_d.ap().rearrange("(t p) j -> p t j", p=128)
                )

            for s in range(1, 4):
                cs = slice(s * 2048, (s + 1) * 2048)
                nc.sync.dma_start(out=x2a[:, cs], in_=x2t_d.ap()[0:128, cs])
                nc.sync.dma_start(out=x2b[0:64, cs], in_=x2t_d.ap()[128:DIN2, cs])

            e1t = big.tile([128, NLOC], BF16)
            e2t = big.tile([128, N], BF16)
            h1t = big.tile([128, 2, NLOC], BF16)
            e1nat = big.tile([128, NT, DF], BF16)
            e2nat = big.tile([128, NT, DF], BF16)
            w1nat = big.tile([128, NT, DF], F32)
            w2snat = big.tile([128, NT, DF], F32)
            w2tnat = w2snat if share_tgt else big.tile([128, NT, DF], F32)
            pos_all = big.tile([128, NT], F32)
            acc_all = big.tile([128, NT * NGRP], F32)
            se_all = big.tile([128, NT], F32)
            # 'd' tiles only write the odd-g slots of acc_all; zero the rest
            nc.gpsimd.memset(acc_all[:], 0.0)

            psA = stack.enter_context(tc.tile_pool(name="psumA", bufs=1, space="PSUM"))

            # ---- PE warm-up: get HAM to 8/8 while input DMAs stream ----
            zr_l = const.tile([128, 128], BF16)
            nc.gpsimd.memset(zr_l[:], 0.0)
            zr_r = const.tile([128, 512], BF16)
            nc.gpsimd.memset(zr_r[:], 0.0)
            warm_ps = psA.tile([128, 512], F32, tag="sps", bufs=2)
            for _ in range(8):
                nc.tensor.matmul(warm_ps[:], zr_l[:], zr_r[:], start=True, stop=True)

            # ---- MLP1: h1T = relu(W1a^T @ x1T + b1a); e1T = W2a^T @ h1T + b2a
            for cc in range(2):
                cols = slice(cc * 512, (cc + 1) * 512)
                h1ps = psA.tile([128, 2, 512], F32, tag="hps", bufs=1)
                for mt in range(2):
                    for kt in range(2):
                        nc.tensor.matmul(
                            h1ps[:, mt, :],
                            w1a[:, kt, mt * 128 : (mt + 1) * 128],
                            x1t[:, kt, cols],
                            start=(kt == 0),
                            stop=(kt == 1),
                        )
                # relu+bias: one on ACT, one on DVE per chunk
                nc.scalar.activation(
                    out=h1t[:, 0, cols],
                    in_=h1ps[:, 0, :],
                    func=mybir.ActivationFunctionType.Relu,
                    bias=b1a[:, 0:1],
                )
                nc.vector.tensor_scalar(
                    out=h1t[:, 1, cols],
                    in0=h1ps[:, 1, :],
                    scalar1=b1a[:, 1:2],
                    scalar2=0.0,
                    op0=mybir.AluOpType.add,
                    op1=mybir.AluOpType.max,
                )
                e1ps = psA.tile([128, 512], F32, tag="sps", bufs=2)
                for kt in range(2):
                    nc.tensor.matmul(
                        e1ps[:],
                        w2a[:, kt, :],
                        h1t[:, kt, cols],
                        start=(kt == 0),
                        stop=(kt == 1),
                    )
                if cc == 0:
                    nc.vector.tensor_scalar_add(
                        out=e1t[:, cols], in0=e1ps[:], scalar1=b2a[:]
                    )
                else:
                    nc.scalar.activation(
                        out=e1t[:, cols],
                        in_=e1ps[:],
                        func=mybir.ActivationFunctionType.Identity,
                        bias=b2a[:],
                    )

            def transpose4(dst, srcT, tt, eng):
                # transpose 4 adjacent 128-col blocks into one PSUM bank,
                # then one 512-col copy out
                tp = psA.tile([128, 4, 128], BF16, tag="sps", bufs=2, name=f"tp{tt}")
                for i in range(4):
                    t = 4 * tt + i
                    nc.tensor.transpose(
                        tp[:, i, :], srcT[:, t * 128 : (t + 1) * 128], ident[:]
                    )
                if eng is nc.scalar:
                    nc.scalar.copy(dst[:, 4 * tt : 4 * tt + 4, :], tp[:])
                else:
                    eng.tensor_copy(dst[:, 4 * tt : 4 * tt + 4, :], tp[:])

            # banded averages (masks prescaled by 1/count):
            # W'[j,:] = sum_{|m-j|<=r} e[m,:] / cnt(j); 16 matmuls into one
            # PSUM bank per batch, one 512-col copy out
            def band_b(dst, bm, src, b, eng):
                wps = psA.tile([128, 4, 128], F32, tag="sps", bufs=2, name=f"wp{b}")
                for jt in range(4):
                    for mt in range(4):
                        nc.tensor.matmul(
                            wps[:, jt, :],
                            bm[:, mt, jt * 128 : (jt + 1) * 128],
                            src[:, 4 * b + mt, :],
                            start=(mt == 0),
                            stop=(mt == 3),
                        )
                if eng is nc.scalar:
                    nc.scalar.copy(dst[:, 4 * b : 4 * b + 4, :], wps[:])
                else:
                    eng.tensor_copy(dst[:, 4 * b : 4 * b + 4, :], wps[:])

            # ---- MLP2 over all N tokens ----
            def mlp2_chunk(ct):
                cols = slice(ct * 512, (ct + 1) * 512)
                h2ps = psA.tile([128, 2, 512], F32, tag="hps", bufs=1, name=f"h2ps{ct}")
                for mt in range(2):
                    msl = slice(mt * 128, (mt + 1) * 128)
                    nc.tensor.matmul(
                        h2ps[:, mt, :], w1b_a[:, msl], x2a[:, cols], start=True, stop=False
                    )
                    nc.tensor.matmul(
                        h2ps[:, mt, :], w1b_b[:, msl], x2b[:, cols], start=False, stop=True
                    )
                h2t = h2pool.tile([128, 2, 512], BF16, tag="h2t", name=f"h2t{ct}")
                # bias already in h2ps (ones-row trick): one 1024-wide relu
                if ct % 2 == 0:
                    nc.scalar.activation(
                        out=h2t[:], in_=h2ps[:], func=mybir.ActivationFunctionType.Relu
                    )
                else:
                    nc.vector.tensor_scalar(
                        out=h2t[:],
                        in0=h2ps[:],
                        scalar1=0.0,
                        scalar2=None,
                        op0=mybir.AluOpType.max,
                    )
                e2ps = psA.tile([128, 512], F32, tag="sps", bufs=2, name=f"e2ps{ct}")
                for kt in range(2):
                    nc.tensor.matmul(
                        e2ps[:], w2b[:, kt, :], h2t[:, kt, :], start=(kt == 0), stop=(kt == 1)
                    )
                nc.vector.tensor_scalar_add(
                    out=e2t[:, cols], in0=e2ps[:], scalar1=b2b[:]
                )

            mlp2_chunk(0)
            mlp2_chunk(1)

            # positives pieces (Pool), consumed inside the group loop:
            # share: pos = e1.(e2 + W1' + W2s') + e2.W2s'
            pos_state = {}

            def make_pos_jobs():
                jobs = []
                for b in range(NB):
                    bsl = slice(4 * b, 4 * b + 4)
                    ga = posp.tile([128, 4, DF], F32, tag=f"posg{b}")
                    gb = posp.tile([128, 4, DF], F32, tag=f"posh{b}")
                    r1 = posp.tile([128, 4], F32, tag=f"post{b}")
                    r2 = posp.tile([128, 4], F32, tag=f"posu{b}")
                    pos_state[b] = (ga, gb, r1, r2)

                    def j1(bsl=bsl, ga=ga):
                        nc.gpsimd.tensor_tensor(
                            out=ga[:], in0=w1nat[:, bsl, :], in1=w2tnat[:, bsl, :],
                            op=mybir.AluOpType.add,
                        )

                    def j2(bsl=bsl, ga=ga):
                        nc.gpsimd.tensor_tensor(
                            out=ga[:], in0=ga[:], in1=e2nat[:, bsl, :],
                            op=mybir.AluOpType.add,
                        )

                    def j3(bsl=bsl, ga=ga):
                        nc.gpsimd.tensor_tensor(
                            out=ga[:], in0=ga[:], in1=e1nat[:, bsl, :],
                            op=mybir.AluOpType.mult,
                        )

                    def j4(bsl=bsl, gb=gb):
                        nc.gpsimd.tensor_tensor(
                            out=gb[:], in0=w2snat[:, bsl, :], in1=e2nat[:, bsl, :],
                            op=mybir.AluOpType.mult,
                        )

                    def j5(b=b, bsl=bsl, ga=ga, gb=gb, r1=r1, r2=r2):
                        nc.vector.tensor_reduce(
                            out=r1[:], in_=ga[:], axis=mybir.AxisListType.X,
                            op=mybir.AluOpType.add,
                        )
                        nc.vector.tensor_reduce(
                            out=r2[:], in_=gb[:], axis=mybir.AxisListType.X,
                            op=mybir.AluOpType.add,
                        )
                        nc.vector.tensor_add(pos_all[:, bsl], r1[:], r2[:])

                    jobs.extend([j1, j2, j3, j4, j5])
                return jobs

            # staged extra work, 2 slots per group: transposes + bands feed
            # the pos jobs; everything here is off the fill/drain critical path
            extra_jobs = [
                lambda: transpose4(e1nat, e1t, 0, nc.vector),
                lambda: transpose4(e1nat, e1t, 1, nc.scalar),
                lambda: band_b(w1nat, bms, e1nat, 0, nc.vector),
                lambda: band_b(w1nat, bms, e1nat, 1, nc.scalar),
                lambda: transpose4(e2nat, e2t, 0, nc.vector),
                lambda: transpose4(e2nat, e2t, 1, nc.scalar),
                lambda: band_b(w2snat, bms, e2nat, 0, nc.vector),
                lambda: band_b(w2snat, bms, e2nat, 1, nc.scalar),
            ]
            if not share_tgt:
                extra_jobs += [
                    lambda: band_b(w2tnat, bmt, e2nat, 0, nc.vector),
                    lambda: band_b(w2tnat, bmt, e2nat, 1, nc.scalar),
                ]
            extra_jobs += make_pos_jobs()

            def emit_extra():
                if extra_jobs:
                    extra_jobs.pop(0)()

            # ---- interleaved MLP2 + negatives, one 1024-col group at a time
            def neg_fill(t, g):
                lhs = e1t[:, t * 128 : (t + 1) * 128]
                np_ps = psA.tile([128, NEG_FD], F32, tag="neg", bufs=2, name=f"np{t}_{g}")
                for i in range(NEG_FD // 512):
                    c0 = g * NEG_FD + i * 512
                    nc.tensor.matmul(
                        np_ps[:, i * 512 : (i + 1) * 512],
                        lhs,
                        e2t[:, c0 : c0 + 512],
                        start=True,
                        stop=True,
                    )
                return np_ps

            ypairs = {}

            def drain(np_ps, t, g):
                idx = t * NGRP + g
                if DRAIN[t] == 'a':
                    nc.scalar.activation(
                        out=np_ps[:],
                        in_=np_ps[:],
                        func=mybir.ActivationFunctionType.Exp,
                        accum_out=acc_all[:, idx : idx + 1],
                    )
                else:
                    if g % 2 == 0:
                        yp = convp.tile(
                            [128, 2, NEG_FD], I16, tag="yp", bufs=4,
                            name=f"yp{t}_{g // 2}",
                        )
                        ypairs[t] = yp
                    else:
                        yp = ypairs[t]
                    nc.vector.tensor_scalar(
                        out=yp[:, g % 2, :],
                        in0=np_ps[:],
                        scalar1=EXP_A2,
                        scalar2=EXP_B2,
                        op0=mybir.AluOpType.mult,
                        op1=mybir.AluOpType.add,
                    )
                    if g % 2 == 1:
                        u = convp.tile(
                            [128, NEG_FD], BF16, tag="u", bufs=2,
                            name=f"u{t}_{g // 2}",
                        )
                        nc.gpsimd.tensor_tensor(
                            out=u[:],
                            in0=yp[:, 0, :].bitcast(BF16),
                            in1=yp[:, 1, :].bitcast(BF16),
                            op=mybir.AluOpType.add,
                        )
                        nc.vector.tensor_reduce(
                            out=acc_all[:, idx : idx + 1],
                            in_=u[:],
                            axis=mybir.AxisListType.X,
                            op=mybir.AluOpType.add,
                        )

            for g in range(NGRP):
                for t in range(4):
                    ps = neg_fill(t, g)
                    drain(ps, t, g)
                if g < NGRP - 1:
                    mlp2_chunk(2 * g + 2)
                emit_extra()
                for t in range(4, NT):
                    ps = neg_fill(t, g)
                    drain(ps, t, g)
                if g < NGRP - 1:
                    mlp2_chunk(2 * g + 3)
                emit_extra()

            while extra_jobs:
                emit_extra()

            nc.sync.dma_start(out=pos_d.ap(), in_=pos_all[:])

            nc.vector.tensor_reduce(
                out=se_all[:],
                in_=acc_all[:].rearrange("p (t g) -> p t g", t=NT),
                axis=mybir.AxisListType.X,
                op=mybir.AluOpType.add,
            )
            nc.sync.dma_start(out=se_d.ap(), in_=se_all[:])

    nc.compile()
    return nc


_BUILD_CACHE: dict = {}


def _get_nc(share_tgt: bool):
    if share_tgt not in _BUILD_CACHE:
        _BUILD_CACHE[share_tgt] = _build(share_tgt)
    return _BUILD_CACHE[share_tgt]


def _band_mask_scaled(r: int) -> np.ndarray:
    """mask[m, j] = 1/cnt(j) if |m-j| <= r (inside [0,L)) else 0."""
    bm = np.zeros((L, L), dtype=np.float32)
    if r > 0:
        j = np.arange(L)
        lo = np.maximum(j - r, 0)
        hi = np.minimum(j + r + 1, L)
        m = np.arange(L)[:, None]
        bm = ((m >= lo[None, :]) & (m < hi[None, :])).astype(np.float32)
        bm /= (hi - lo).astype(np.float32)[None, :]
    return bm


def kernel(**inputs):
    loss, _ = _run(inputs, trace=False)
    return loss


def _run(inputs, trace=False, trace_kwargs=None):
    import ml_dtypes

    bf16 = ml_dtypes.bfloat16
    feature1 = inputs["feature1"]
    feature2 = inputs["feature2"]
    W1a, b1a, W2a, b2a = inputs["W1a"], inputs["b1a"], inputs["W2a"], inputs["b2a"]
    W1b, b1b, W2b, b2b = inputs["W1b"], inputs["b1b"], inputs["W2b"], inputs["b2b"]
    f1 = np.ascontiguousarray(np.asarray(feature1, dtype=np.float32))
    f2 = np.ascontiguousarray(np.asarray(feature2, dtype=np.float32))
    r_self = int(np.asarray(inputs["positive_range_self"]))
    r_tgt = int(np.asarray(inputs["positive_range_tgt"]))
    share_tgt = r_tgt == r_self

    nc = _get_nc(share_tgt)

    x2t_full = np.ascontiguousarray(f2.reshape(N, DIN2).T.astype(bf16))  # (192, 8192)
    common = {
        "w1a": np.ascontiguousarray(np.asarray(W1a, np.float32).astype(bf16)),
        "w2a": np.ascontiguousarray(np.asarray(W2a, np.float32).astype(bf16)),
        "w1b": np.ascontiguousarray(np.asarray(W1b, np.float32).astype(bf16)),
        "w2b": np.ascontiguousarray(np.asarray(W2b, np.float32).astype(bf16)),
        "b1a": np.ascontiguousarray(np.asarray(b1a, np.float32).reshape(2, 128).T),
        "b2a": np.asarray(b2a, np.float32).reshape(128, 1),
        "b1b_row": np.asarray(b1b, np.float32).reshape(1, DH).astype(bf16),
        "b2b": np.asarray(b2b, np.float32).reshape(128, 1),
        "bms": _band_mask_scaled(r_self).astype(bf16),
    }
    if not share_tgt:
        common["bmt"] = _band_mask_scaled(r_tgt).astype(bf16)

    in_maps = []
    for c in range(NCORES):
        x1t = np.ascontiguousarray(
            f1[c * NB : (c + 1) * NB].reshape(NLOC, DIN1).T.astype(bf16)
        )  # (256, 1024)
        # rotate feature2^T columns so this core's rows come first
        x2t = np.ascontiguousarray(
            np.concatenate(
                [x2t_full[:, c * NLOC :], x2t_full[:, : c * NLOC]], axis=1
            )
        )
        in_maps.append({**common, "x1t": x1t, "x2t": x2t})

    res = run_bass_kernel_spmd(
        nc,
        in_maps,
        core_ids=list(range(NCORES)),
        trace=trace,
        **(trace_kwargs or {}),
    )

    pos = np.empty(N, dtype=np.float64)
    se = np.empty(N, dtype=np.float64)
    for c in range(NCORES):
        # column t holds local rows [t*128, (t+1)*128) in partitions
        p = res.results[c]["pos_out"]  # (128, NT)
        s = res.results[c]["se_out"]
        pos[c * NLOC : (c + 1) * NLOC] = p.T.reshape(NLOC)
        se[c * NLOC : (c + 1) * NLOC] = s.T.reshape(NLOC)

    neg = np.log(se) - np.log(float(N))
    loss = np.mean(-pos + neg)
    return np.array(loss, dtype=np.float32), res


# revision 13
# speedup vs baseline: 1.2644x; 1.0601x over previous
"""Contrastive-learning loss kernel for Trainium2 (8 NeuronCores, Bass/Tile).

Problem (hardcoded shapes): B=16, L=512, DIN1=256, DIN2=192, DH=256, DF=128.
  emb1 = MLP_a(feature1); emb2 = MLP_b(feature2)          # (B, L, DF)
  positive = rowdot(f1, f2) + band-mean terms              # (N,)  N = B*L = 8192
  negative = logsumexp(f1 @ f2.T, axis=-1) - log N         # (N,)
  loss = mean(-positive + negative)

Sharding: data-parallel over B for embeddings/positives (2 batches per core);
the N x N negatives matrix is sharded row-wise. Each core computes the full
emb2 from a column-ROTATED copy of feature2 (its own batches first), so the
device program is identical across cores (pure SPMD, no partition-id): the
local rows are always columns [0, 1024) and logsumexp is invariant to column
order.

v3: the per-core wall is the PSUM drain of the sim tiles (only ACT and DVE
can read PSUM, both at ~1 elem/cycle/lane). Changes vs v2:
  - 3-way drain: ACT does exp with fused accum (4 tiles/group); DVE does the
    Schraudolph int16 convert only (4 tiles/group); GPSIMD picks up the bf16
    row-sums of the DVE tiles via tensor_scalar(accum_out=...) from SBUF
    (~1 cyc/elem, it cannot read PSUM but the y16 tiles are in SBUF).
  - MLP2's b1b is folded into the h2 matmul (x2b carries a ones row, w1b_b a
    bias row), so the h2 drain is ONE 1024-wide relu per chunk instead of two
    biased 512-wide tensor_scalars.
  - everything bf16 (weights, h, e) halves SBUF and enables FWL.
  - transposes/bands/pos pieces are staged as jobs consumed inside the group
    loop so the first neg fill starts right after MLP1 + 2 MLP2 chunks.
Band masks arrive prescaled by 1/count so the banded-mean rescale is free:
  pos = e1.(e2 + W1' + W2t') + e2.W2s'
"""

import numpy as np

import concourse.bacc as bacc
import concourse.tile as tile
from concourse import mybir
from concourse.bass_utils import run_bass_kernel_spmd
from concourse.masks import make_identity

F32 = mybir.dt.float32
BF16 = mybir.dt.bfloat16
I16 = mybir.dt.int16

B, L, DIN1, DIN2, DH, DF = 16, 512, 256, 192, 256, 128
N = B * L            # 8192 total rows
NCORES = 8
NB = B // NCORES     # 2 local batches per core
NLOC = NB * L        # 1024 local rows per core
NT = NLOC // 128     # 8 local row tiles
NEG_FD = 1024        # columns drained per instruction
NGRP = N // NEG_FD   # 8 column groups

# Schraudolph fast-exp in int16/bf16: bf16_bits(exp(x)) ~= int16(A2*x + B2)
# (output cast truncates; sigma tuned for ~zero mean rel err, max ~4%)
EXP_A2 = float(2.0**7 / np.log(2.0))
EXP_B2 = float(127.0 * 2.0**7 - 6.833)

# Per-group drain of the 8 row tiles: 'a' = ACT exp + fused accum,
# 'd' = DVE Schraudolph convert; the two converts of a group PAIR are
# then combined with one bf16 2x-mode add + one reduce (per-elem reduce
# work halves vs a reduce per group).
DRAIN = ['a', 'd', 'a', 'a', 'd', 'a', 'd', 'a']


def _build(share_tgt: bool):
    nc = bacc.Bacc("TRN2", target_bir_lowering=False, debug=False)

    x1t_d = nc.dram_tensor("x1t", [DIN1, NLOC], BF16, kind="ExternalInput")
    x2t_d = nc.dram_tensor("x2t", [DIN2, N], BF16, kind="ExternalInput")
    w1a_d = nc.dram_tensor("w1a", [DIN1, DH], BF16, kind="ExternalInput")
    w2a_d = nc.dram_tensor("w2a", [DH, DF], BF16, kind="ExternalInput")
    w1b_d = nc.dram_tensor("w1b", [DIN2, DH], BF16, kind="ExternalInput")
    w2b_d = nc.dram_tensor("w2b", [DH, DF], BF16, kind="ExternalInput")
    b1a_d = nc.dram_tensor("b1a", [128, 2], F32, kind="ExternalInput")
    b2a_d = nc.dram_tensor("b2a", [128, 1], F32, kind="ExternalInput")
    b1b_d = nc.dram_tensor("b1b_row", [1, DH], BF16, kind="ExternalInput")
    b2b_d = nc.dram_tensor("b2b", [128, 1], F32, kind="ExternalInput")
    # band masks prescaled by 1/count(j)
    bms_d = nc.dram_tensor("bms", [L, L], BF16, kind="ExternalInput")
    if not share_tgt:
        bmt_d = nc.dram_tensor("bmt", [L, L], BF16, kind="ExternalInput")
    pos_d = nc.dram_tensor("pos_out", [128, NT], F32, kind="ExternalOutput")
    se_d = nc.dram_tensor("se_out", [128, NT], F32, kind="ExternalOutput")

    with tile.TileContext(nc) as tc:
        import contextlib

        with contextlib.ExitStack() as stack:
            const = stack.enter_context(tc.tile_pool(name="const", bufs=1))
            big = stack.enter_context(tc.tile_pool(name="big", bufs=1))
            h2pool = stack.enter_context(tc.tile_pool(name="h2pool", bufs=3))
            posp = stack.enter_context(tc.tile_pool(name="posp", bufs=1))
            convp = stack.enter_context(tc.tile_pool(name="convp", bufs=3))

            # ---- constants / inputs (ordered so MLP1 can start ASAP) ----
            ident = const.tile([128, 128], BF16)
            make_identity(nc, ident[:])

            # sync queue: MLP1's critical inputs first (x1t gates everything)
            x1t = big.tile([128, 2, NLOC], BF16)
            for cc in range(2):
                nc.sync.dma_start(
                    out=x1t[:, :, cc * 512 : (cc + 1) * 512],
                    in_=x1t_d.ap().rearrange("(t p) c -> p t c", p=128)[
                        :, :, cc * 512 : (cc + 1) * 512
                    ],
                )
            x2a = big.tile([128, N], BF16)
            x2b = big.tile([65, N], BF16)
            nc.gpsimd.memset(x2b[64:65, :], 1.0)  # ones row: h2 += 1*b1b
            nc.sync.dma_start(out=x2a[:, 0:2048], in_=x2t_d.ap()[0:128, 0:2048])
            nc.sync.dma_start(out=x2b[0:64, 0:2048], in_=x2t_d.ap()[128:DIN2, 0:2048])
            w1a = const.tile([128, 2, DH], BF16)
            nc.sync.dma_start(
                out=w1a[:], in_=w1a_d.ap().rearrange("(t p) m -> p t m", p=128)
            )
            b1a = const.tile([128, 2], F32)
            nc.sync.dma_start(out=b1a[:], in_=b1a_d.ap())
            w2a = const.tile([128, 2, DF], BF16)
            nc.sync.dma_start(
                out=w2a[:], in_=w2a_d.ap().rearrange("(t p) m -> p t m", p=128)
            )
            b2a = const.tile([128, 1], F32)
            nc.sync.dma_start(out=b2a[:], in_=b2a_d.ap())

            # scalar (ACT) HWDGE queue in parallel: MLP2 weights + x2 slab 0
            w1b_a = const.tile([128, DH], BF16)
            nc.scalar.dma_start(out=w1b_a[:], in_=w1b_d.ap()[0:128, :])
            # augmented: rows 0-63 = w1b[128:192], row 64 = b1b (ones-row trick)
            w1b_b = const.tile([65, DH], BF16)
            nc.scalar.dma_start(out=w1b_b[0:64, :], in_=w1b_d.ap()[128:192, :])
            nc.scalar.dma_start(out=w1b_b[64:65, :], in_=b1b_d.ap())
            w2b = const.tile([128, 2, DF], BF16)
            nc.scalar.dma_start(
                out=w2b[:], in_=w2b_d.ap().rearrange("(t p) m -> p t m", p=128)
            )
            b2b = const.tile([128, 1], F32)
            nc.scalar.dma_start(out=b2b[:], in_=b2b_d.ap())

            bms = const.tile([128, 4, L], BF16)
            nc.scalar.dma_start(
                out=bms[:], in_=bms_d.ap().rearrange("(t p) j -> p t j", p=128)
            )
            if share_tgt:
                bmt = bms
            else:
                bmt = const.tile([128, 4, L], BF16)
                nc.scalar.dma_start(
                    out=bmt[:], in_=bmt_d.ap().rearrange("(t p) j -> p t j", p=128)
                )

            for s in range(1, 4):
                cs = slice(s * 2048, (s + 1) * 2048)
                nc.sync.dma_start(out=x2a[:, cs], in_=x2t_d.ap()[0:128, cs])
                nc.sync.dma_start(out=x2b[0:64, cs], in_=x2t_d.ap()[128:DIN2, cs])

            e1t = big.tile([128, NLOC], BF16)
            e2t = big.tile([128, N], BF16)
            h1t = big.tile([128, 2, NLOC], BF16)
            e1nat = big.tile([128, NT, DF], BF16)
            e2nat = big.tile([128, NT, DF], BF16)
            w1nat = big.tile([128, NT, DF], F32)
            w2snat = big.tile([128, NT, DF], F32)
            w2tnat = w2snat if share_tgt else big.tile([128, NT, DF], F32)
            pos_all = big.tile([128, NT], F32)
            acc_all = big.tile([128, NT * NGRP], F32)
            se_all = big.tile([128, NT], F32)
            # 'd' tiles only write the odd-g slots of acc_all; zero the rest
            nc.gpsimd.memset(acc_all[:], 0.0)

            psA = stack.enter_context(tc.tile_pool(name="psumA", bufs=1, space="PSUM"))

            # ---- PE warm-up: get HAM to 8/8 while input DMAs stream ----
            zr_l = const.tile([128, 128], BF16)
            nc.gpsimd.memset(zr_l[:], 0.0)
            zr_r = const.tile([128, 512], BF16)
            nc.gpsimd.memset(zr_r[:], 0.0)
            warm_ps = psA.tile([128, 512], F32, tag="sps", bufs=2)
            for _ in range(8):
                nc.tensor.matmul(warm_ps[:], zr_l[:], zr_r[:], start=True, stop=True)

            # ---- MLP1: h1T = relu(W1a^T @ x1T + b1a); e1T = W2a^T @ h1T + b2a
            for cc in range(2):
                cols = slice(cc * 512, (cc + 1) * 512)
                h1ps = psA.tile([128, 2, 512], F32, tag="hps", bufs=1)
                for mt in range(2):
                    for kt in range(2):
                        nc.tensor.matmul(
                            h1ps[:, mt, :],
                            w1a[:, kt, mt * 128 : (mt + 1) * 128],
                            x1t[:, kt, cols],
                            start=(kt == 0),
                            stop=(kt == 1),
                        )
                # relu+bias: one on ACT, one on DVE per chunk
                nc.scalar.activation(
                    out=h1t[:, 0, cols],
                    in_=h1ps[:, 0, :],
                    func=mybir.ActivationFunctionType.Relu,
                    bias=b1a[:, 0:1],
                )
                nc.vector.tensor_scalar(
                    out=h1t[:, 1, cols],
                    in0=h1ps[:, 1, :],
                    scalar1=b1a[:, 1:2],
                    scalar2=0.0,
                    op0=mybir.AluOpType.add,
                    op1=mybir.AluOpType.max,
                )
                e1ps = psA.tile([128, 512], F32, tag="sps", bufs=2)
                for kt in range(2):
                    nc.tensor.matmul(
                        e1ps[:],
                        w2a[:, kt, :],
                        h1t[:, kt, cols],
                        start=(kt == 0),
                        stop=(kt == 1),
                    )
                if cc == 0:
                    nc.vector.tensor_scalar_add(
                        out=e1t[:, cols], in0=e1ps[:], scalar1=b2a[:]
                    )
                else:
                    nc.scalar.activation(
                        out=e1t[:, cols],
                        in_=e1ps[:],
                        func=mybir.ActivationFunctionType.Identity,
                        bias=b2a[:],
                    )

            def transpose4(dst, srcT, tt, eng):
                # transpose 4 adjacent 128-col blocks into one PSUM bank,
                # then one 512-col copy out
                tp = psA.tile([128, 4, 128], BF16, tag="sps", bufs=2, name=f"tp{tt}")
                for i in range(4):
                    t = 4 * tt + i
                    nc.tensor.transpose(
                        tp[:, i, :], srcT[:, t * 128 : (t + 1) * 128], ident[:]
                    )
                if eng is nc.scalar:
                    nc.scalar.copy(dst[:, 4 * tt : 4 * tt + 4, :], tp[:])
                else:
                    eng.tensor_copy(dst[:, 4 * tt : 4 * tt + 4, :], tp[:])

            # banded averages (masks prescaled by 1/count):
            # W'[j,:] = sum_{|m-j|<=r} e[m,:] / cnt(j); 16 matmuls into one
            # PSUM bank per batch, one 512-col copy out
            def band_b(dst, bm, src, b, eng):
                wps = psA.tile([128, 4, 128], F32, tag="sps", bufs=2, name=f"wp{b}")
                for jt in range(4):
                    for mt in range(4):
                        nc.tensor.matmul(
                            wps[:, jt, :],
                            bm[:, mt, jt * 128 : (jt + 1) * 128],
                            src[:, 4 * b + mt, :],
                            start=(mt == 0),
                            stop=(mt == 3),
                        )
                if eng is nc.scalar:
                    nc.scalar.copy(dst[:, 4 * b : 4 * b + 4, :], wps[:])
                else:
                    eng.tensor_copy(dst[:, 4 * b : 4 * b + 4, :], wps[:])

            # ---- MLP2 over all N tokens ----
            def mlp2_chunk(ct):
                cols = slice(ct * 512, (ct + 1) * 512)
                h2ps = psA.tile([128, 2, 512], F32, tag="hps", bufs=1, name=f"h2ps{ct}")
                for mt in range(2):
                    msl = slice(mt * 128, (mt + 1) * 128)
                    nc.tensor.matmul(
                        h2ps[:, mt, :], w1b_a[:, msl], x2a[:, cols], start=True, stop=False
                    )
                    nc.tensor.matmul(
                        h2ps[:, mt, :], w1b_b[:, msl], x2b[:, cols], start=False, stop=True
                    )
                h2t = h2pool.tile([128, 2, 512], BF16, tag="h2t", name=f"h2t{ct}")
                # bias already in h2ps (ones-row trick): one 1024-wide relu
                if ct % 2 == 0:
                    nc.scalar.activation(
                        out=h2t[:], in_=h2ps[:], func=mybir.ActivationFunctionType.Relu
                    )
                else:
                    nc.vector.tensor_scalar(
                        out=h2t[:],
                        in0=h2ps[:],
                        scalar1=0.0,
                        scalar2=None,
                        op0=mybir.AluOpType.max,
                    )
                e2ps = psA.tile([128, 512], F32, tag="sps", bufs=2, name=f"e2ps{ct}")
                for kt in range(2):
                    nc.tensor.matmul(
                        e2ps[:], w2b[:, kt, :], h2t[:, kt, :], start=(kt == 0), stop=(kt == 1)
                    )
                if ct % 2 == 0:
                    nc.vector.tensor_scalar_add(
                        out=e2t[:, cols], in0=e2ps[:], scalar1=b2b[:]
                    )
                else:
                    nc.scalar.activation(
                        out=e2t[:, cols],
                        in_=e2ps[:],
                        func=mybir.ActivationFunctionType.Identity,
                        bias=b2b[:],
                    )

            mlp2_chunk(0)
            mlp2_chunk(1)

            # positives pieces (Pool), consumed inside the group loop:
            # share: pos = e1.(e2 + W1' + W2s') + e2.W2s'
            pos_state = {}

            def make_pos_jobs():
                jobs = []
                for b in range(NB):
                    bsl = slice(4 * b, 4 * b + 4)
                    ga = posp.tile([128, 4, DF], F32, tag=f"posg{b}")
                    gb = posp.tile([128, 4, DF], F32, tag=f"posh{b}")
                    r1 = posp.tile([128, 4], F32, tag=f"post{b}")
                    r2 = posp.tile([128, 4], F32, tag=f"posu{b}")
                    pos_state[b] = (ga, gb, r1, r2)

                    def j1(bsl=bsl, ga=ga):
                        nc.gpsimd.tensor_tensor(
                            out=ga[:], in0=w1nat[:, bsl, :], in1=w2tnat[:, bsl, :],
                            op=mybir.AluOpType.add,
                        )

                    def j2(bsl=bsl, ga=ga):
                        nc.gpsimd.tensor_tensor(
                            out=ga[:], in0=ga[:], in1=e2nat[:, bsl, :],
                            op=mybir.AluOpType.add,
                        )

                    def j3(bsl=bsl, ga=ga):
                        nc.gpsimd.tensor_tensor(
                            out=ga[:], in0=ga[:], in1=e1nat[:, bsl, :],
                            op=mybir.AluOpType.mult,
                        )

                    def j4(bsl=bsl, gb=gb):
                        nc.gpsimd.tensor_tensor(
                            out=gb[:], in0=w2snat[:, bsl, :], in1=e2nat[:, bsl, :],
                            op=mybir.AluOpType.mult,
                        )

                    jobs.extend([j1, j2, j3, j4])
                return jobs

            # staged extra work, 2 slots per group: transposes + bands feed
            # the pos jobs; everything here is off the fill/drain critical path
            extra_jobs = [
                lambda: transpose4(e1nat, e1t, 0, nc.vector),
                lambda: transpose4(e1nat, e1t, 1, nc.scalar),
                lambda: band_b(w1nat, bms, e1nat, 0, nc.vector),
                lambda: band_b(w1nat, bms, e1nat, 1, nc.scalar),
                lambda: transpose4(e2nat, e2t, 0, nc.vector),
                lambda: transpose4(e2nat, e2t, 1, nc.scalar),
                lambda: band_b(w2snat, bms, e2nat, 0, nc.vector),
                lambda: band_b(w2snat, bms, e2nat, 1, nc.scalar),
            ]
            if not share_tgt:
                extra_jobs += [
                    lambda: band_b(w2tnat, bmt, e2nat, 0, nc.vector),
                    lambda: band_b(w2tnat, bmt, e2nat, 1, nc.scalar),
                ]
            extra_jobs += make_pos_jobs()

            def emit_extra():
                if extra_jobs:
                    extra_jobs.pop(0)()

            # ---- interleaved MLP2 + negatives, one 1024-col group at a time
            def neg_fill(t, g):
                lhs = e1t[:, t * 128 : (t + 1) * 128]
                np_ps = psA.tile([128, NEG_FD], F32, tag="neg", bufs=2, name=f"np{t}_{g}")
                for i in range(NEG_FD // 512):
                    c0 = g * NEG_FD + i * 512
                    nc.tensor.matmul(
                        np_ps[:, i * 512 : (i + 1) * 512],
                        lhs,
                        e2t[:, c0 : c0 + 512],
                        start=True,
                        stop=True,
                    )
                return np_ps

            ypairs = {}

            def drain(np_ps, t, g):
                idx = t * NGRP + g
                if DRAIN[t] == 'a':
                    nc.scalar.activation(
                        out=np_ps[:],
                        in_=np_ps[:],
                        func=mybir.ActivationFunctionType.Exp,
                        accum_out=acc_all[:, idx : idx + 1],
                    )
                else:
                    if g % 2 == 0:
                        yp = convp.tile(
                            [128, 2, NEG_FD], I16, tag="yp", bufs=4,
                            name=f"yp{t}_{g // 2}",
                        )
                        ypairs[t] = yp
                    else:
                        yp = ypairs[t]
                    nc.vector.tensor_scalar(
                        out=yp[:, g % 2, :],
                        in0=np_ps[:],
                        scalar1=EXP_A2,
                        scalar2=EXP_B2,
                        op0=mybir.AluOpType.mult,
                        op1=mybir.AluOpType.add,
                    )
                    if g % 2 == 1:
                        u = convp.tile(
                            [128, NEG_FD], BF16, tag="u", bufs=2,
                            name=f"u{t}_{g // 2}",
                        )
                        nc.vector.tensor_tensor(
                            out=u[:],
                            in0=yp[:, 0, :].bitcast(BF16),
                            in1=yp[:, 1, :].bitcast(BF16),
                            op=mybir.AluOpType.add,
                        )
                        nc.vector.tensor_reduce(
                            out=acc_all[:, idx : idx + 1],
                            in_=u[:],
                            axis=mybir.AxisListType.X,
                            op=mybir.AluOpType.add,
                        )

            for g in range(NGRP):
                for t in range(4):
                    ps = neg_fill(t, g)
                    drain(ps, t, g)
                if g < NGRP - 1:
                    mlp2_chunk(2 * g + 2)
                emit_extra()
                for t in range(4, NT):
                    ps = neg_fill(t, g)
                    drain(ps, t, g)
                if g < NGRP - 1:
                    mlp2_chunk(2 * g + 3)
                emit_extra()

            while extra_jobs:
                emit_extra()

            # pos tail: reduces + final combine on DVE
            for b in range(NB):
                bsl = slice(4 * b, 4 * b + 4)
                ga, gb, r1, r2 = pos_state[b]
                nc.vector.tensor_reduce(
                    out=r1[:], in_=ga[:], axis=mybir.AxisListType.X, op=mybir.AluOpType.add
                )
                nc.vector.tensor_reduce(
                    out=r2[:], in_=gb[:], axis=mybir.AxisListType.X, op=mybir.AluOpType.add
                )
                nc.vector.tensor_add(pos_all[:, bsl], r1[:], r2[:])
            nc.sync.dma_start(out=pos_d.ap(), in_=pos_all[:])

            nc.vector.tensor_reduce(
                out=se_all[:],
                in_=acc_all[:].rearrange("p (t g) -> p t g", t=NT),
                axis=mybir.AxisListType.X,
                op=mybir.AluOpType.add,
            )
            nc.sync.dma_start(out=se_d.ap(), in_=se_all[:])

    nc.compile()
    return nc


_BUILD_CACHE: dict = {}


def _get_nc(share_tgt: bool):
    if share_tgt not in _BUILD_CACHE:
        _BUILD_CACHE[share_tgt] = _build(share_tgt)
    return _BUILD_CACHE[share_tgt]


def _band_mask_scaled(r: int) -> np.ndarray:
    """mask[m, j] = 1/cnt(j) if |m-j| <= r (inside [0,L)) else 0."""
    bm = np.zeros((L, L), dtype=np.float32)
    if r > 0:
        j = np.arange(L)
        lo = np.maximum(j - r, 0)
        hi = np.minimum(j + r + 1, L)
        m = np.arange(L)[:, None]
        bm = ((m >= lo[None, :]) & (m < hi[None, :])).astype(np.float32)
        bm /= (hi - lo).astype(np.float32)[None, :]
    return bm


def kernel(**inputs):
    loss, _ = _run(inputs, trace=False)
    return loss


def _run(inputs, trace=False, trace_kwargs=None):
    import ml_dtypes

    bf16 = ml_dtypes.bfloat16
    feature1 = inputs["feature1"]
    feature2 = inputs["feature2"]
    W1a, b1a, W2a, b2a = inputs["W1a"], inputs["b1a"], inputs["W2a"], inputs["b2a"]
    W1b, b1b, W2b, b2b = inputs["W1b"], inputs["b1b"], inputs["W2b"], inputs["b2b"]
    f1 = np.ascontiguousarray(np.asarray(feature1, dtype=np.float32))
    f2 = np.ascontiguousarray(np.asarray(feature2, dtype=np.float32))
    r_self = int(np.asarray(inputs["positive_range_self"]))
    r_tgt = int(np.asarray(inputs["positive_range_tgt"]))
    share_tgt = r_tgt == r_self

    nc = _get_nc(share_tgt)

    x2t_full = np.ascontiguousarray(f2.reshape(N, DIN2).T.astype(bf16))  # (192, 8192)
    common = {
        "w1a": np.ascontiguousarray(np.asarray(W1a, np.float32).astype(bf16)),
        "w2a": np.ascontiguousarray(np.asarray(W2a, np.float32).astype(bf16)),
        "w1b": np.ascontiguousarray(np.asarray(W1b, np.float32).astype(bf16)),
        "w2b": np.ascontiguousarray(np.asarray(W2b, np.float32).astype(bf16)),
        "b1a": np.ascontiguousarray(np.asarray(b1a, np.float32).reshape(2, 128).T),
        "b2a": np.asarray(b2a, np.float32).reshape(128, 1),
        "b1b_row": np.asarray(b1b, np.float32).reshape(1, DH).astype(bf16),
        "b2b": np.asarray(b2b, np.float32).reshape(128, 1),
        "bms": _band_mask_scaled(r_self).astype(bf16),
    }
    if not share_tgt:
        common["bmt"] = _band_mask_scaled(r_tgt).astype(bf16)

    in_maps = []
    for c in range(NCORES):
        x1t = np.ascontiguousarray(
            f1[c * NB : (c + 1) * NB].reshape(NLOC, DIN1).T.astype(bf16)
        )  # (256, 1024)
        # rotate feature2^T columns so this core's rows come first
        x2t = np.ascontiguousarray(
            np.concatenate(
                [x2t_full[:, c * NLOC :], x2t_full[:, : c * NLOC]], axis=1
            )
        )
        in_maps.append({**common, "x1t": x1t, "x2t": x2t})

    res = run_bass_kernel_spmd(
        nc,
        in_maps,
        core_ids=list(range(NCORES)),
        trace=trace,
        **(trace_kwargs or {}),
    )

    pos = np.empty(N, dtype=np.float64)
    se = np.empty(N, dtype=np.float64)
    for c in range(NCORES):
        # column t holds local rows [t*128, (t+1)*128) in partitions
        p = res.results[c]["pos_out"]  # (128, NT)
        s = res.results[c]["se_out"]
        pos[c * NLOC : (c + 1) * NLOC] = p.T.reshape(NLOC)
        se[c * NLOC : (c + 1) * NLOC] = s.T.reshape(NLOC)

    neg = np.log(se) - np.log(float(N))
    loss = np.mean(-pos + neg)
    return np.array(loss, dtype=np.float32), res
